# revision 8
# baseline (speedup 1.0000x reference)
"""Trainium2 Bass kernel for AntecedentShareGMF (fuzzy rule softmax).

Math: given X [N, D], center/sigma [D, M] with M=2, R = M^D = 1024 rules
(rule r selects per-feature MF index i(r,d) = bit (D-1-d) of r):
    z[n, r] = (1/D) * sum_d -0.5 * (X[n,d] - C[r,d])^2 / (S[r,d]^2 + eps)
    out = softmax_r(z)

The quadratic expands into z = Xext @ W where Xext = [X, X^2, 1] ([N, 2D+1])
and W [2D+1, R] is built on-device from center/sigma:
    W[d,     r] = -2 * w[r,d] * C[r,d]      (coeff of X[n,d])
    W[D+d,   r] = w[r,d]                    (coeff of X[n,d]^2)
    W[2D,    r] = sum_d w[r,d] * C[r,d]^2   (constant)
    w[r,d] = -0.5 / D / (sigma[d,i]^2 + eps)

Data-parallel over N across 8 cores; center/sigma replicated; softmax over R
is local per sample (free axis), so no cross-core communication.
"""

import numpy as np

import concourse.bass as bass
import concourse.bacc as bacc
import concourse.tile as tile
from concourse import mybir
from concourse.bass_utils import run_bass_kernel_spmd
from concourse.masks import make_identity

N, D, M = 8192, 10, 2
R = M**D  # 1024
NCORES = 8
NSHARD = N // NCORES  # 1024
P = 128
NTILES = NSHARD // P  # 8
EPS = 1e-8
F32 = mybir.dt.float32
HR = 512  # half of R; one PSUM bank / max matmul free size
AF = mybir.ActivationFunctionType
ALU = mybir.AluOpType


def build_nc() -> bass.Bass:
    nc = bacc.Bacc()
    X = nc.declare_dram_parameter("X", [NSHARD, D], F32, isOutput=False)
    center = nc.declare_dram_parameter("center", [D, M], F32, isOutput=False)
    sigma = nc.declare_dram_parameter("sigma", [D, M], F32, isOutput=False)
    out = nc.declare_dram_parameter("out", [NSHARD, R], F32, isOutput=True)

    with tile.TileContext(nc) as tc:
        with (
            tc.tile_pool(name="consts", bufs=1) as consts,
            tc.tile_pool(name="xe", bufs=3) as xe_pool,
            tc.tile_pool(name="xt", bufs=3) as xt_pool,
            tc.tile_pool(name="prob", bufs=4) as prob_pool,
            tc.tile_pool(name="stat", bufs=8) as stat_pool,
            tc.tile_pool(name="pt", bufs=2, space="PSUM") as pt_pool,
            tc.tile_pool(name="pc", bufs=1, space="PSUM") as pc_pool,
            tc.tile_pool(name="pz", bufs=2, space="PSUM") as pz_pool,
        ):
            ident = consts.tile([P, P], F32)
            make_identity(nc, ident)

            cen = consts.tile([D, M], F32)
            sig = consts.tile([D, M], F32)
            nc.sync.dma_start(out=cen, in_=center[:, :])
            nc.sync.dma_start(out=sig, in_=sigma[:, :])

            # w01[d, m] = (-0.5/D) / (sigma[d,m]^2 + eps)
            epsb = consts.tile([D, 1], F32)
            nc.vector.memset(epsb, EPS)
            sq = consts.tile([D, M], F32)
            nc.vector.tensor_mul(out=sq, in0=sig, in1=sig)
            nc.vector.tensor_scalar_add(out=sq, in0=sq, scalar1=epsb)
            w01 = consts.tile([D, M], F32)
            nc.vector.reciprocal(out=w01, in_=sq)
            nc.scalar.mul(out=w01, in_=w01, mul=-0.5 / D)

            wdiff = consts.tile([D, 1], F32)
            nc.vector.tensor_sub(out=wdiff, in0=w01[:, 1:2], in1=w01[:, 0:1])
            cdiff = consts.tile([D, 1], F32)
            nc.vector.tensor_sub(out=cdiff, in0=cen[:, 1:2], in1=cen[:, 0:1])

            # B[d, r] = bit (D-1-d) of r, as f32: iota r per row, shift right
            # by (D-1-d) per-partition, mask bit 0. Engine-op APs must start
            # at partition 0, so everything is built as full start-0 tiles.
            it = consts.tile([D, R], mybir.dt.int32)
            nc.gpsimd.iota(out=it, pattern=[[1, R]], base=0, channel_multiplier=0)
            sh = consts.tile([D, 1], mybir.dt.int32)
            nc.gpsimd.iota(out=sh, pattern=[[0, 1]], base=D - 1, channel_multiplier=-1)
            nc.vector.tensor_scalar(
                out=it, in0=it, scalar1=sh, scalar2=1,
                op0=ALU.arith_shift_right, op1=ALU.bitwise_and,
            )
            B = consts.tile([D, R], F32)
            nc.vector.tensor_copy(out=B, in_=it)

            W = consts.tile([2 * D + 1, R], F32)
            # w table (X^2 coeffs); lands in W rows D..2D-1 via DMA below
            Wa = consts.tile([D, R], F32)
            nc.vector.tensor_scalar(
                out=Wa, in0=B, scalar1=wdiff, scalar2=w01[:, 0:1],
                op0=ALU.mult, op1=ALU.add,
            )
            # per-rule centers C[d, r]
            Ct = consts.tile([D, R], F32)
            nc.vector.tensor_scalar(
                out=Ct, in0=B, scalar1=cdiff, scalar2=cen[:, 0:1],
                op0=ALU.mult, op1=ALU.add,
            )
            wc = consts.tile([D, R], F32)
            nc.vector.tensor_mul(out=wc, in0=Wa, in1=Ct)
            # rows 0..D-1: -2*w*C (X coeffs); partition start 0, write in place
            nc.scalar.mul(out=W[0:D, :], in_=wc, mul=-2.0)
            # row 2D: sum_d w*C^2 via ones-matmul partition reduction
            V = consts.tile([D, R], F32)
            nc.vector.tensor_mul(out=V, in0=wc, in1=Ct)
            ones_d = consts.tile([D, 1], F32)
            nc.vector.memset(ones_d, 1.0)
            pc = pc_pool.tile([1, R], F32)
            nc.tensor.matmul(out=pc[:, 0:HR], lhsT=ones_d, rhs=V[:, 0:HR])
            nc.tensor.matmul(out=pc[:, HR:R], lhsT=ones_d, rhs=V[:, HR:R])
            Wc = consts.tile([1, R], F32)
            nc.vector.tensor_copy(out=Wc, in_=pc[0:1, :])
            # assemble W rows D..2D via DMA (no partition-alignment limits)
            nc.sync.dma_start(out=W[D : 2 * D, :], in_=Wa)
            nc.sync.dma_start(out=W[2 * D : 2 * D + 1, :], in_=Wc)

            for t in range(NTILES):
                xe = xe_pool.tile([P, 2 * D + 1], F32)
                nc.sync.dma_start(out=xe[:, 0:D], in_=X[t * P : (t + 1) * P, :])
                nc.scalar.activation(out=xe[:, D : 2 * D], in_=xe[:, 0:D], func=AF.Square)
                nc.vector.memset(xe[:, 2 * D : 2 * D + 1], 1.0)

                pt = pt_pool.tile([2 * D + 1, P], F32)
                nc.tensor.transpose(out=pt, in_=xe, identity=ident)
                xt = xt_pool.tile([2 * D + 1, P], F32)
                nc.vector.tensor_copy(out=xt, in_=pt)

                pz = pz_pool.tile([P, R], F32)
                nc.tensor.matmul(out=pz[:, 0:HR], lhsT=xt, rhs=W[:, 0:HR])
                nc.tensor.matmul(out=pz[:, HR:R], lhsT=xt, rhs=W[:, HR:R])

                negmax = stat_pool.tile([P, 1], F32)
                nc.vector.reduce_max(
                    out=negmax, in_=pz, axis=mybir.AxisListType.X, negate=True
                )
                prob = prob_pool.tile([P, R], F32)
                sums = stat_pool.tile([P, 1], F32)
                nc.scalar.activation(
                    out=prob, in_=pz, func=AF.Exp, bias=negmax, scale=1.0,
                    accum_out=sums,
                )
                rsum = stat_pool.tile([P, 1], F32)
                nc.vector.reciprocal(out=rsum, in_=sums)
                nc.vector.tensor_scalar_mul(out=prob, in0=prob, scalar1=rsum)
                nc.sync.dma_start(out=out[t * P : (t + 1) * P, :], in_=prob)

    return nc


_NC_CACHE: list = []


def _get_nc() -> bass.Bass:
    if not _NC_CACHE:
        nc = build_nc()
        if not nc.is_finalized():
            nc.finalize()  # runs Bacc.compile (wait splitting, reg alloc)
        _NC_CACHE.append(nc)
    return _NC_CACHE[0]


def run(X, center, sigma, **spmd_kwargs):
    X = np.ascontiguousarray(np.asarray(X, dtype=np.float32))
    center = np.ascontiguousarray(np.asarray(center, dtype=np.float32))
    sigma = np.ascontiguousarray(np.asarray(sigma, dtype=np.float32))
    nc = _get_nc()
    in_maps = [
        {"X": X[i * NSHARD : (i + 1) * NSHARD], "center": center, "sigma": sigma}
        for i in range(NCORES)
    ]
    res = run_bass_kernel_spmd(nc, in_maps, core_ids=list(range(NCORES)), **spmd_kwargs)
    out = np.concatenate(
        [np.asarray(res.results[i]["out"]) for i in range(NCORES)], axis=0
    )
    return out, res


def kernel(**inputs) -> np.ndarray:
    out, _ = run(inputs["X"], inputs["center"], inputs["sigma"])
    return out
